# revision 10
# baseline (speedup 1.0000x reference)
"""Trainium2 Bass kernel for BaselineDNN: embedding gather + length-normalized
sum-pool over L tokens + 2-layer MLP.

  logits[b] = relu((sum_l emb[x[b,l]]) / len[b] @ W1 + b1) @ W2 + b2

Sharding: data-parallel over batch. Each of the 8 cores handles B/8 = 256
batch rows; the embedding table and tiny MLP weights are replicated.

Gather strategy: the HW dma_gather primitive takes int16 (signed) row
indices, so a 50000-row table is unreachable in one gather. The table is
padded/shifted on host into [50002, 320] f32 (row 0 = zeros, rows
1..50000 = emb, row 50001 = zeros) and every token is routed to one of two
base views:
  lo: rows [0, 32768)      idx = x + 1       (x <= 32766), filler idx 0
  hi: rows [17234, 50002)  idx = x - 17233   (x >= 32767), filler idx 32767
Per batch row the 200 tokens are partitioned (host-side, order-invariant
under the sum) into a lo-list and a hi-list, padded with zero-row fillers
to the global maxima K_LO / K_HI. Each 128-row batch tile then issues
ceil(K_LO/C) + ceil(K_HI/C) rectangular dma_gathers of up to C=50 tokens
per row (one 1280B row per index).

Pooling: per gathered token-slot, a TensorE matmul with a 128x128 identity
as the stationary operand accumulates the [128, 300] slot slice into PSUM
(acc += I.T @ g_slot). Then ScalarE scales by 1/len, TensorE transposes
rep and runs the two MLP layers, ScalarE applies bias/relu. Logits are
written transposed [3, 256] per core; the host reassembles [2048, 3].
"""

import numpy as np
from contextlib import ExitStack

import concourse.bass as bass
import concourse.bacc as bacc
import concourse.mybir as mybir
import concourse.tile as tile
from concourse.bass_utils import run_bass_kernel_spmd
from concourse.masks import make_identity

# Problem shapes (hardcoded per spec)
B, L, V, D, H, C = 2048, 200, 50000, 300, 32, 3
N_CORES = 8
BS = B // N_CORES   # 256 batch rows per core
P = 128             # partitions
N_TILES = BS // P   # batch tiles per core
E = 320             # padded embedding row (1280B, multiple of 256B)
CHUNK = 25          # max tokens per dma_gather (128*25=3200 descriptors
                    # fits the per-engine SWDGE ring: ~256 descs x 16 engines)
D_CHUNKS = [(0, 128), (128, 128), (256, 44)]  # D=300 split for transposes

LO_BASE = 0         # lo view: table rows [0, 32768)
HI_BASE = 17234     # hi view: table rows [17234, 50002)
LO_FILL = 0         # zero row (table row 0)
HI_FILL = 32767     # zero row (table row 50001)
X_SPLIT = 32767     # x < split -> lo (idx x+1); x >= split -> hi (idx x-17233)

F32 = mybir.dt.float32
I16 = mybir.dt.int16

_CACHE = {}


def _chunks(total):
    out = []
    while total > 0:
        c = min(CHUNK, total)
        out.append(c)
        total -= c
    return out


def _build_nc(k_lo, k_hi):
    lo_chunks = _chunks(k_lo)
    hi_chunks = _chunks(k_hi)
    n_slots = k_lo + k_hi
    idx_cols = 8 * n_slots  # wrapped idx layout: 16 rows per 128 indices

    nc = bacc.Bacc("TRN2", debug=False, num_devices=N_CORES)

    idx_in = nc.declare_dram_parameter("idx", [N_TILES, P, idx_cols], I16,
                                       isOutput=False)
    len_in = nc.declare_dram_parameter("lens", [BS, 1], F32, isOutput=False)
    emb_in = nc.declare_dram_parameter("emb", [V + 2, E], F32, isOutput=False)
    w1_in = nc.declare_dram_parameter("w1", [D, H], F32, isOutput=False)
    b1_in = nc.declare_dram_parameter("b1", [H, 1], F32, isOutput=False)
    w2_in = nc.declare_dram_parameter("w2", [H, C], F32, isOutput=False)
    b2_in = nc.declare_dram_parameter("b2", [C, 1], F32, isOutput=False)
    out_dram = nc.declare_dram_parameter("out", [C, BS], F32, isOutput=True)

    emb_lo = emb_in[LO_BASE:LO_BASE + 32768, :]
    emb_hi = emb_in[HI_BASE:HI_BASE + 32768, :]

    with tile.TileContext(nc) as tc, ExitStack() as ctx:
        const_pool = ctx.enter_context(tc.tile_pool(name="const", bufs=1))
        xpool = ctx.enter_context(tc.tile_pool(name="xp", bufs=2))
        gpool = ctx.enter_context(tc.tile_pool(name="gp", bufs=2))
        spool = ctx.enter_context(tc.tile_pool(name="sp", bufs=2))
        psum_pool = ctx.enter_context(tc.tile_pool(name="ps", bufs=2, space="PSUM"))
        psum_acc = ctx.enter_context(tc.tile_pool(name="psacc", bufs=2, space="PSUM"))

        ident = const_pool.tile([P, P], F32)
        make_identity(nc, ident[:])
        w1_sb = const_pool.tile([P, 3 * H], F32)  # chunk j at cols [j*H, (j+1)*H)
        for j, (d0, dc) in enumerate(D_CHUNKS):
            nc.sync.dma_start(w1_sb[:dc, j * H:(j + 1) * H], w1_in[d0:d0 + dc, :])
        b1_sb = const_pool.tile([H, 1], F32)
        nc.sync.dma_start(b1_sb[:], b1_in[:])
        w2_sb = const_pool.tile([H, C], F32)
        nc.sync.dma_start(w2_sb[:], w2_in[:])
        b2_sb = const_pool.tile([C, 1], F32)
        nc.sync.dma_start(b2_sb[:], b2_in[:])

        for t in range(N_TILES):
            r0 = t * P
            idx_t = xpool.tile([P, idx_cols], I16, tag="xt")
            nc.sync.dma_start(idx_t[:], idx_in[t, :, :])
            lens_t = xpool.tile([P, 1], F32, tag="lt")
            nc.sync.dma_start(lens_t[:], len_in[r0:r0 + P, :])
            inv_t = xpool.tile([P, 1], F32, tag="it")
            nc.vector.reciprocal(inv_t[:], lens_t[:])

            acc = psum_acc.tile([P, D], F32)
            n_done = 0  # token slots accumulated so far
            col0 = 0    # idx column offset
            plan = [(c, emb_lo) for c in lo_chunks] + [(c, emb_hi) for c in hi_chunks]
            for ci, (c, src) in enumerate(plan):
                g = gpool.tile([P, CHUNK * E], F32, tag="g")
                gv = g[:, :c * E].rearrange("p (c e) -> p c e", c=c, e=E)
                nc.gpsimd.dma_gather(
                    out_ap=gv,
                    in_ap=src,
                    idxs_ap=idx_t[:, col0:col0 + 8 * c],
                    num_idxs=P * c,
                    num_idxs_reg=P * c,
                    elem_size=E,
                    # >64 descriptors per engine: must not coalesce the whole
                    # stream into one SDMA packet (64-descriptor HW ceiling)
                    single_packet=False,
                )
                col0 += 8 * c
                for k in range(c):
                    nc.tensor.matmul(
                        out=acc[:],
                        lhsT=ident[:],
                        rhs=gv[:, k, 0:D],
                        start=(n_done == 0),
                        stop=(n_done == n_slots - 1),
                    )
                    n_done += 1

            # rep = acc / len  (ScalarE: PSUM -> SBUF with per-partition scale)
            rep = spool.tile([P, D], F32, tag="rep")
            nc.scalar.mul(rep[:], acc[:], inv_t[:, :1])

            # repT chunks + first MLP layer: h = relu(rep @ W1 + b1), as [H, P]
            h_psum = psum_pool.tile([H, P], F32, tag="h")
            for j, (d0, dc) in enumerate(D_CHUNKS):
                tp = psum_pool.tile([P, P], F32, tag="tp")
                nc.tensor.transpose(tp[:dc, :], rep[:, d0:d0 + dc], ident[:])
                repT = spool.tile([P, P], F32, tag="repT")
                nc.vector.tensor_copy(repT[:dc, :], tp[:dc, :])
                nc.tensor.matmul(
                    out=h_psum[:],
                    lhsT=w1_sb[:dc, j * H:(j + 1) * H],
                    rhs=repT[:dc, :],
                    start=(j == 0),
                    stop=(j == len(D_CHUNKS) - 1),
                )
            h_sb = spool.tile([H, P], F32, tag="hsb")
            nc.scalar.activation(
                h_sb[:], h_psum[:], mybir.ActivationFunctionType.Relu,
                bias=b1_sb[:, :1], scale=1.0,
            )

            # logits = h @ W2 + b2, as [C, P]
            o_psum = psum_pool.tile([C, P], F32, tag="o")
            nc.tensor.matmul(out=o_psum[:], lhsT=w2_sb[:], rhs=h_sb[:],
                             start=True, stop=True)
            logits_sb = spool.tile([C, P], F32, tag="lg")
            nc.scalar.activation(
                logits_sb[:], o_psum[:], mybir.ActivationFunctionType.Identity,
                bias=b2_sb[:, :1], scale=1.0,
            )
            nc.sync.dma_start(out_dram[:, r0:r0 + P], logits_sb[:])

    nc.finalize()
    return nc


def _wrap_idx(block):
    """[P, C] token-slot indices -> [P, 8*C] wrapped+replicated int16 tile.

    dma_gather maps flat index j -> partition j%128, column-group j//128, and
    reads the flat list wrapped over 16 partitions (element j at partition
    j%16, column j//16), replicated across the eight 16-partition groups.
    """
    p, c = block.shape
    flat = block.T.reshape(-1)              # j = col*128 + p order
    w = flat.reshape(8 * c, 16).T           # [16, 8*c]: element j at (j%16, j//16)
    return np.tile(w, (8, 1))               # replicate to 128 partitions


def _prep_idx(x32):
    """Split tokens lo/hi per row, pad with fillers, build wrapped idx tiles.

    Returns (idx arrays per core [N_TILES, P, 8*(k_lo+k_hi)], k_lo, k_hi).
    """
    is_hi = x32 >= X_SPLIT
    n_lo = (~is_hi).sum(axis=1)                      # [B]
    k_lo = int(n_lo.max())
    k_hi = int((L - n_lo).max())
    order = np.argsort(is_hi, axis=1, kind="stable")  # lo positions first
    xo = np.take_along_axis(x32, order, axis=1)       # [B, L] lo tokens then hi

    cols = np.arange(L)[None, :]
    lo_vals = np.where(cols < n_lo[:, None], xo + 1, LO_FILL)
    # hi tokens start at column n_lo[p]
    hi_src = np.take_along_axis(
        xo, np.minimum(cols + n_lo[:, None], L - 1), axis=1)
    hi_vals = np.where(cols < (L - n_lo)[:, None], hi_src - 17233, HI_FILL)

    lo16 = lo_vals[:, :k_lo].astype(np.int16)
    hi16 = hi_vals[:, :k_hi].astype(np.int16)

    idx_per_core = []
    for c in range(N_CORES):
        tiles = []
        for t in range(N_TILES):
            r0 = c * BS + t * P
            blocks = []
            for c0 in range(0, k_lo, CHUNK):
                blocks.append(_wrap_idx(lo16[r0:r0 + P, c0:min(c0 + CHUNK, k_lo)]))
            for c0 in range(0, k_hi, CHUNK):
                blocks.append(_wrap_idx(hi16[r0:r0 + P, c0:min(c0 + CHUNK, k_hi)]))
            tiles.append(np.concatenate(blocks, axis=1))
        idx_per_core.append(np.ascontiguousarray(np.stack(tiles)))
    return idx_per_core, k_lo, k_hi


def _prep_inputs(x, lengths, emb_table, W1, b1, W2, b2):
    x32 = np.asarray(x).astype(np.int32)
    idx_per_core, k_lo, k_hi = _prep_idx(x32)

    lens = np.ascontiguousarray(
        np.asarray(lengths).astype(np.float32).reshape(B, 1))
    emb_p = np.zeros((V + 2, E), dtype=np.float32)
    emb_p[1:V + 1, :D] = np.asarray(emb_table, dtype=np.float32)
    w1 = np.ascontiguousarray(np.asarray(W1, dtype=np.float32))
    b1c = np.ascontiguousarray(np.asarray(b1, dtype=np.float32).reshape(H, 1))
    w2 = np.ascontiguousarray(np.asarray(W2, dtype=np.float32))
    b2c = np.ascontiguousarray(np.asarray(b2, dtype=np.float32).reshape(C, 1))
    in_maps = [
        {
            "idx": idx_per_core[c],
            "lens": lens[c * BS:(c + 1) * BS],
            "emb": emb_p,
            "w1": w1,
            "b1": b1c,
            "w2": w2,
            "b2": b2c,
        }
        for c in range(N_CORES)
    ]
    return in_maps, k_lo, k_hi


def run_on_device(in_maps, k_lo, k_hi, **kwargs):
    key = (k_lo, k_hi)
    if _CACHE.get("key") != key:
        _CACHE["nc"] = _build_nc(k_lo, k_hi)
        _CACHE["key"] = key
    return run_bass_kernel_spmd(_CACHE["nc"], in_maps, list(range(N_CORES)),
                                **kwargs)


def kernel(x, lengths, emb_table, W1, b1, W2, b2):
    in_maps, k_lo, k_hi = _prep_inputs(x, lengths, emb_table, W1, b1, W2, b2)
    res = run_on_device(in_maps, k_lo, k_hi)
    out = np.concatenate([r["out"] for r in res.results], axis=1)  # [C, B]
    return np.ascontiguousarray(out.T).astype(np.float32)  # [B, C]


# revision 12
# speedup vs baseline: 140.4840x; 140.4840x over previous
"""Trainium2 Bass kernel for BaselineDNN: embedding gather + length-normalized
sum-pool over L tokens + 2-layer MLP.

  logits[b] = relu((sum_l emb[x[b,l]]) / len[b] @ W1 + b1) @ W2 + b2

Sharding: data-parallel over batch. Each of the 8 cores handles B/8 = 256
batch rows; the embedding table and tiny MLP weights are replicated.

Gather strategy: the HW dma_gather primitive takes int16 (signed) row
indices, so a 50000-row table is unreachable in one gather. The table is
padded/shifted on host into [50002, 320] f32 (row 0 = zeros, rows
1..50000 = emb, row 50001 = zeros) and every token is routed to one of two
base views:
  lo: rows [0, 32768)      idx = x + 1       (x <= 32766), filler idx 0
  hi: rows [17234, 50002)  idx = x - 17233   (x >= 32767), filler idx 32767
Per batch row the 200 tokens are partitioned (host-side, order-invariant
under the sum) into a lo-list and a hi-list, padded with zero-row fillers
to the global maxima K_LO / K_HI. Each 128-row batch tile then issues
ceil(K_LO/C) + ceil(K_HI/C) rectangular dma_gathers of up to C=50 tokens
per row (one 1280B row per index).

Pooling: per gathered token-slot, a TensorE matmul with a 128x128 identity
as the stationary operand accumulates the [128, 300] slot slice into PSUM
(acc += I.T @ g_slot). Then ScalarE scales by 1/len, TensorE transposes
rep and runs the two MLP layers, ScalarE applies bias/relu. Logits are
written transposed [3, 256] per core; the host reassembles [2048, 3].
"""

import numpy as np
from contextlib import ExitStack

import concourse.bass as bass
import concourse.bacc as bacc
import concourse.mybir as mybir
import concourse.tile as tile
from concourse.bass_utils import run_bass_kernel_spmd
from concourse.masks import make_identity

# Problem shapes (hardcoded per spec)
B, L, V, D, H, C = 2048, 200, 50000, 300, 32, 3
N_CORES = 8
BS = B // N_CORES   # 256 batch rows per core
P = 128             # partitions
N_TILES = BS // P   # batch tiles per core
E = 320             # padded embedding row (1280B, multiple of 256B)
CHUNK = 25          # max tokens per dma_gather (128*25=3200 descriptors
                    # fits the per-engine SWDGE ring: ~256 descs x 16 engines)
D_CHUNKS = [(0, 128), (128, 128), (256, 44)]  # D=300 split for transposes

LO_BASE = 0         # lo view: table rows [0, 32768)
HI_BASE = 17234     # hi view: table rows [17234, 50002)
LO_FILL = 0         # zero row (table row 0)
HI_FILL = 32767     # zero row (table row 50001)
X_SPLIT = 32767     # x < split -> lo (idx x+1); x >= split -> hi (idx x-17233)

F32 = mybir.dt.float32
I16 = mybir.dt.int16

_CACHE = {}


def _chunks(total):
    out = []
    while total > 0:
        c = min(CHUNK, total)
        out.append(c)
        total -= c
    return out


def _build_nc(k_lo, k_hi, reps=1):
    lo_chunks = _chunks(k_lo)
    hi_chunks = _chunks(k_hi)
    n_slots = k_lo + k_hi
    idx_cols = 8 * n_slots  # wrapped idx layout: 16 rows per 128 indices

    nc = bacc.Bacc("TRN2", debug=False, num_devices=N_CORES)

    idx_in = nc.declare_dram_parameter("idx", [N_TILES, P, idx_cols], I16,
                                       isOutput=False)
    len_in = nc.declare_dram_parameter("lens", [BS, 1], F32, isOutput=False)
    emb_in = nc.declare_dram_parameter("emb", [V + 2, E], F32, isOutput=False)
    w1_in = nc.declare_dram_parameter("w1", [D, H], F32, isOutput=False)
    b1_in = nc.declare_dram_parameter("b1", [H, 1], F32, isOutput=False)
    w2_in = nc.declare_dram_parameter("w2", [H, C], F32, isOutput=False)
    b2_in = nc.declare_dram_parameter("b2", [C, 1], F32, isOutput=False)
    out_dram = nc.declare_dram_parameter("out", [C, BS], F32, isOutput=True)

    emb_lo = emb_in[LO_BASE:LO_BASE + 32768, :]
    emb_hi = emb_in[HI_BASE:HI_BASE + 32768, :]

    with tile.TileContext(nc) as tc, ExitStack() as ctx:
        const_pool = ctx.enter_context(tc.tile_pool(name="const", bufs=1))
        xpool = ctx.enter_context(tc.tile_pool(name="xp", bufs=2))
        gpool = ctx.enter_context(tc.tile_pool(name="gp", bufs=2))
        spool = ctx.enter_context(tc.tile_pool(name="sp", bufs=2))
        psum_pool = ctx.enter_context(tc.tile_pool(name="ps", bufs=2, space="PSUM"))
        psum_acc = ctx.enter_context(tc.tile_pool(name="psacc", bufs=2, space="PSUM"))

        ident = const_pool.tile([P, P], F32)
        make_identity(nc, ident[:])
        w1_sb = const_pool.tile([P, 3 * H], F32)  # chunk j at cols [j*H, (j+1)*H)
        for j, (d0, dc) in enumerate(D_CHUNKS):
            nc.sync.dma_start(w1_sb[:dc, j * H:(j + 1) * H], w1_in[d0:d0 + dc, :])
        b1_sb = const_pool.tile([H, 1], F32)
        nc.sync.dma_start(b1_sb[:], b1_in[:])
        w2_sb = const_pool.tile([H, C], F32)
        nc.sync.dma_start(w2_sb[:], w2_in[:])
        b2_sb = const_pool.tile([C, 1], F32)
        nc.sync.dma_start(b2_sb[:], b2_in[:])

        loop_ctx = tc.For_i(0, reps, 1) if reps > 1 else None
        if loop_ctx is not None:
            ctx.enter_context(loop_ctx)

        for t in range(N_TILES):
            r0 = t * P
            idx_t = xpool.tile([P, idx_cols], I16, tag="xt")
            nc.sync.dma_start(idx_t[:], idx_in[t, :, :])
            lens_t = xpool.tile([P, 1], F32, tag="lt")
            nc.sync.dma_start(lens_t[:], len_in[r0:r0 + P, :])
            inv_t = xpool.tile([P, 1], F32, tag="it")
            nc.vector.reciprocal(inv_t[:], lens_t[:])

            acc = psum_acc.tile([P, D], F32)
            n_done = 0  # token slots accumulated so far
            col0 = 0    # idx column offset
            plan = [(c, emb_lo) for c in lo_chunks] + [(c, emb_hi) for c in hi_chunks]
            for ci, (c, src) in enumerate(plan):
                g = gpool.tile([P, CHUNK * E], F32, tag="g")
                gv = g[:, :c * E].rearrange("p (c e) -> p c e", c=c, e=E)
                nc.gpsimd.dma_gather(
                    out_ap=gv,
                    in_ap=src,
                    idxs_ap=idx_t[:, col0:col0 + 8 * c],
                    num_idxs=P * c,
                    num_idxs_reg=P * c,
                    elem_size=E,
                    # >64 descriptors per engine: must not coalesce the whole
                    # stream into one SDMA packet (64-descriptor HW ceiling)
                    single_packet=False,
                )
                col0 += 8 * c
                for k in range(c):
                    nc.tensor.matmul(
                        out=acc[:],
                        lhsT=ident[:],
                        rhs=gv[:, k, 0:D],
                        start=(n_done == 0),
                        stop=(n_done == n_slots - 1),
                    )
                    n_done += 1

            # rep = acc / len  (ScalarE: PSUM -> SBUF with per-partition scale)
            rep = spool.tile([P, D], F32, tag="rep")
            nc.scalar.mul(rep[:], acc[:], inv_t[:, :1])

            # repT chunks + first MLP layer: h = relu(rep @ W1 + b1), as [H, P]
            h_psum = psum_pool.tile([H, P], F32, tag="h")
            for j, (d0, dc) in enumerate(D_CHUNKS):
                tp = psum_pool.tile([P, P], F32, tag="tp")
                nc.tensor.transpose(tp[:dc, :], rep[:, d0:d0 + dc], ident[:])
                repT = spool.tile([P, P], F32, tag="repT")
                nc.vector.tensor_copy(repT[:dc, :], tp[:dc, :])
                nc.tensor.matmul(
                    out=h_psum[:],
                    lhsT=w1_sb[:dc, j * H:(j + 1) * H],
                    rhs=repT[:dc, :],
                    start=(j == 0),
                    stop=(j == len(D_CHUNKS) - 1),
                )
            h_sb = spool.tile([H, P], F32, tag="hsb")
            nc.scalar.activation(
                h_sb[:], h_psum[:], mybir.ActivationFunctionType.Relu,
                bias=b1_sb[:, :1], scale=1.0,
            )

            # logits = h @ W2 + b2, as [C, P]
            o_psum = psum_pool.tile([C, P], F32, tag="o")
            nc.tensor.matmul(out=o_psum[:], lhsT=w2_sb[:], rhs=h_sb[:],
                             start=True, stop=True)
            logits_sb = spool.tile([C, P], F32, tag="lg")
            nc.scalar.activation(
                logits_sb[:], o_psum[:], mybir.ActivationFunctionType.Identity,
                bias=b2_sb[:, :1], scale=1.0,
            )
            nc.sync.dma_start(out_dram[:, r0:r0 + P], logits_sb[:])

    nc.finalize()
    return nc


def _wrap_idx(block):
    """[P, C] token-slot indices -> [P, 8*C] wrapped+replicated int16 tile.

    dma_gather maps flat index j -> partition j%128, column-group j//128, and
    reads the flat list wrapped over 16 partitions (element j at partition
    j%16, column j//16), replicated across the eight 16-partition groups.
    """
    p, c = block.shape
    flat = block.T.reshape(-1)              # j = col*128 + p order
    w = flat.reshape(8 * c, 16).T           # [16, 8*c]: element j at (j%16, j//16)
    return np.tile(w, (8, 1))               # replicate to 128 partitions


def _prep_idx(x32):
    """Split tokens lo/hi per row, pad with fillers, build wrapped idx tiles.

    Returns (idx arrays per core [N_TILES, P, 8*(k_lo+k_hi)], k_lo, k_hi).
    """
    is_hi = x32 >= X_SPLIT
    n_lo = (~is_hi).sum(axis=1)                      # [B]
    k_lo = int(n_lo.max())
    k_hi = int((L - n_lo).max())
    order = np.argsort(is_hi, axis=1, kind="stable")  # lo positions first
    xo = np.take_along_axis(x32, order, axis=1)       # [B, L] lo tokens then hi

    cols = np.arange(L)[None, :]
    lo_vals = np.where(cols < n_lo[:, None], xo + 1, LO_FILL)
    # hi tokens start at column n_lo[p]
    hi_src = np.take_along_axis(
        xo, np.minimum(cols + n_lo[:, None], L - 1), axis=1)
    hi_vals = np.where(cols < (L - n_lo)[:, None], hi_src - 17233, HI_FILL)

    lo16 = lo_vals[:, :k_lo].astype(np.int16)
    hi16 = hi_vals[:, :k_hi].astype(np.int16)

    idx_per_core = []
    for c in range(N_CORES):
        tiles = []
        for t in range(N_TILES):
            r0 = c * BS + t * P
            blocks = []
            for c0 in range(0, k_lo, CHUNK):
                blocks.append(_wrap_idx(lo16[r0:r0 + P, c0:min(c0 + CHUNK, k_lo)]))
            for c0 in range(0, k_hi, CHUNK):
                blocks.append(_wrap_idx(hi16[r0:r0 + P, c0:min(c0 + CHUNK, k_hi)]))
            tiles.append(np.concatenate(blocks, axis=1))
        idx_per_core.append(np.ascontiguousarray(np.stack(tiles)))
    return idx_per_core, k_lo, k_hi


def _prep_inputs(x, lengths, emb_table, W1, b1, W2, b2):
    x32 = np.asarray(x).astype(np.int32)
    idx_per_core, k_lo, k_hi = _prep_idx(x32)

    lens = np.ascontiguousarray(
        np.asarray(lengths).astype(np.float32).reshape(B, 1))
    emb_p = np.zeros((V + 2, E), dtype=np.float32)
    emb_p[1:V + 1, :D] = np.asarray(emb_table, dtype=np.float32)
    w1 = np.ascontiguousarray(np.asarray(W1, dtype=np.float32))
    b1c = np.ascontiguousarray(np.asarray(b1, dtype=np.float32).reshape(H, 1))
    w2 = np.ascontiguousarray(np.asarray(W2, dtype=np.float32))
    b2c = np.ascontiguousarray(np.asarray(b2, dtype=np.float32).reshape(C, 1))
    in_maps = [
        {
            "idx": idx_per_core[c],
            "lens": lens[c * BS:(c + 1) * BS],
            "emb": emb_p,
            "w1": w1,
            "b1": b1c,
            "w2": w2,
            "b2": b2c,
        }
        for c in range(N_CORES)
    ]
    return in_maps, k_lo, k_hi


def run_on_device(in_maps, k_lo, k_hi, **kwargs):
    key = (k_lo, k_hi)
    if _CACHE.get("key") != key:
        _CACHE["nc"] = _build_nc(k_lo, k_hi)
        _CACHE["key"] = key
    return run_bass_kernel_spmd(_CACHE["nc"], in_maps, list(range(N_CORES)),
                                **kwargs)


def kernel(x, lengths, emb_table, W1, b1, W2, b2):
    in_maps, k_lo, k_hi = _prep_inputs(x, lengths, emb_table, W1, b1, W2, b2)
    res = run_on_device(in_maps, k_lo, k_hi)
    out = np.concatenate([r["out"] for r in res.results], axis=1)  # [C, B]
    return np.ascontiguousarray(out.T).astype(np.float32)  # [B, C]


# revision 16
# speedup vs baseline: 173.3261x; 1.2338x over previous
"""Trainium2 Bass kernel for BaselineDNN: embedding gather + length-normalized
sum-pool over L tokens + 2-layer MLP.

  logits[b] = relu((sum_l emb[x[b,l]]) / len[b] @ W1 + b1) @ W2 + b2

Sharding: data-parallel over batch. Each of the 8 cores handles B/8 = 256
batch rows; the embedding table and tiny MLP weights are replicated.

Gather strategy: the HW dma_gather primitive takes int16 (signed) row
indices, so a 50000-row table is unreachable in one gather. The table is
padded/shifted on host into [50002, 320] f32 (row 0 = zeros, rows
1..50000 = emb, row 50001 = zeros) and every token is routed to one of two
base views:
  lo: rows [0, 32768)      idx = x + 1       (x <= 32766), filler idx 0
  hi: rows [17234, 50002)  idx = x - 17233   (x >= 32767), filler idx 32767
Per batch row the 200 tokens are partitioned (host-side, order-invariant
under the sum) into a lo-list and a hi-list, padded with zero-row fillers
to the global maxima K_LO / K_HI. Each 128-row batch tile then issues
ceil(K_LO/C) + ceil(K_HI/C) rectangular dma_gathers of up to C=50 tokens
per row (one 1280B row per index).

Pooling: per gathered token-slot, a TensorE matmul with a 128x128 identity
as the stationary operand accumulates the [128, 300] slot slice into PSUM
(acc += I.T @ g_slot). Then ScalarE scales by 1/len, TensorE transposes
rep and runs the two MLP layers, ScalarE applies bias/relu. Logits are
written transposed [3, 256] per core; the host reassembles [2048, 3].
"""

import numpy as np
from contextlib import ExitStack

import concourse.bass as bass
import concourse.bacc as bacc
import concourse.mybir as mybir
import concourse.tile as tile
from concourse.bass_utils import run_bass_kernel_spmd
from concourse.masks import make_identity

# Problem shapes (hardcoded per spec)
B, L, V, D, H, C = 2048, 200, 50000, 300, 32, 3
N_CORES = 8
BS = B // N_CORES   # 256 batch rows per core
P = 128             # partitions
N_TILES = BS // P   # batch tiles per core
E = 320             # padded embedding row (1280B, multiple of 256B)
CHUNK = 25          # max tokens per dma_gather (128*25=3200 descriptors
                    # fits the per-engine SWDGE ring: ~256 descs x 16 engines)
D_CHUNKS = [(0, 128), (128, 128), (256, 44)]  # D=300 split for transposes

LO_BASE = 0         # lo view: table rows [0, 32768)
HI_BASE = 17234     # hi view: table rows [17234, 50002)
LO_FILL = 0         # zero row (table row 0)
HI_FILL = 32767     # zero row (table row 50001)
X_SPLIT = 32767     # x < split -> lo (idx x+1); x >= split -> hi (idx x-17233)

F32 = mybir.dt.float32
I16 = mybir.dt.int16

_CACHE = {}


def _chunks(total):
    out = []
    while total > 0:
        c = min(CHUNK, total)
        out.append(c)
        total -= c
    return out


def _build_nc(k_lo, k_hi, reps=1):
    lo_chunks = _chunks(k_lo)
    hi_chunks = _chunks(k_hi)
    n_slots = k_lo + k_hi
    idx_cols = 8 * n_slots  # wrapped idx layout: 16 rows per 128 indices

    # 4 SWDGE queues: a single queue serializes gathers on per-queue ring
    # bookkeeping (await prior DMA completion); round-robin over 4 queues
    # keeps descriptor-gen and transfers pipelined at HBM bandwidth.
    nc = bacc.Bacc("TRN2", debug=False, num_devices=N_CORES,
                   num_swdge_queues=4, dynamic_dma_scratch_size=32768)

    idx_in = nc.declare_dram_parameter("idx", [N_TILES, P, idx_cols], I16,
                                       isOutput=False)
    len_in = nc.declare_dram_parameter("lens", [BS, 1], F32, isOutput=False)
    emb_in = nc.declare_dram_parameter("emb", [V + 2, E], F32, isOutput=False)
    w1_in = nc.declare_dram_parameter("w1", [D, H], F32, isOutput=False)
    b1_in = nc.declare_dram_parameter("b1", [H, 1], F32, isOutput=False)
    w2_in = nc.declare_dram_parameter("w2", [H, C], F32, isOutput=False)
    b2_in = nc.declare_dram_parameter("b2", [C, 1], F32, isOutput=False)
    out_dram = nc.declare_dram_parameter("out", [C, BS], F32, isOutput=True)

    emb_lo = emb_in[LO_BASE:LO_BASE + 32768, :]
    emb_hi = emb_in[HI_BASE:HI_BASE + 32768, :]

    with tile.TileContext(nc) as tc, ExitStack() as ctx:
        const_pool = ctx.enter_context(tc.tile_pool(name="const", bufs=1))
        xpool = ctx.enter_context(tc.tile_pool(name="xp", bufs=2))
        gpool = ctx.enter_context(tc.tile_pool(name="gp", bufs=4))
        spool = ctx.enter_context(tc.tile_pool(name="sp", bufs=2))
        psum_pool = ctx.enter_context(tc.tile_pool(name="ps", bufs=2, space="PSUM"))
        psum_acc = ctx.enter_context(tc.tile_pool(name="psacc", bufs=2, space="PSUM"))

        ident = const_pool.tile([P, P], F32)
        make_identity(nc, ident[:])
        w1_sb = const_pool.tile([P, 3 * H], F32)  # chunk j at cols [j*H, (j+1)*H)
        for j, (d0, dc) in enumerate(D_CHUNKS):
            nc.sync.dma_start(w1_sb[:dc, j * H:(j + 1) * H], w1_in[d0:d0 + dc, :])
        b1_sb = const_pool.tile([H, 1], F32)
        nc.sync.dma_start(b1_sb[:], b1_in[:])
        w2_sb = const_pool.tile([H, C], F32)
        nc.sync.dma_start(w2_sb[:], w2_in[:])
        b2_sb = const_pool.tile([C, 1], F32)
        nc.sync.dma_start(b2_sb[:], b2_in[:])

        loop_ctx = tc.For_i(0, reps, 1) if reps > 1 else None
        if loop_ctx is not None:
            ctx.enter_context(loop_ctx)

        for t in range(N_TILES):
            r0 = t * P
            idx_t = xpool.tile([P, idx_cols], I16, tag="xt")
            nc.sync.dma_start(idx_t[:], idx_in[t, :, :])
            lens_t = xpool.tile([P, 1], F32, tag="lt")
            nc.sync.dma_start(lens_t[:], len_in[r0:r0 + P, :])
            inv_t = xpool.tile([P, 1], F32, tag="it")
            nc.vector.reciprocal(inv_t[:], lens_t[:])

            acc = psum_acc.tile([P, D], F32)
            n_done = 0  # token slots accumulated so far
            col0 = 0    # idx column offset
            plan = [(c, emb_lo) for c in lo_chunks] + [(c, emb_hi) for c in hi_chunks]
            for ci, (c, src) in enumerate(plan):
                g = gpool.tile([P, CHUNK * E], F32, tag="g")
                gv = g[:, :c * E].rearrange("p (c e) -> p c e", c=c, e=E)
                nc.gpsimd.dma_gather(
                    out_ap=gv,
                    in_ap=src,
                    idxs_ap=idx_t[:, col0:col0 + 8 * c],
                    num_idxs=P * c,
                    num_idxs_reg=P * c,
                    elem_size=E,
                    # >64 descriptors per engine: must not coalesce the whole
                    # stream into one SDMA packet (64-descriptor HW ceiling)
                    single_packet=False,
                    queue_num=(t * len(plan) + ci) % 4,
                )
                col0 += 8 * c
                for k in range(c):
                    nc.tensor.matmul(
                        out=acc[:],
                        lhsT=ident[:],
                        rhs=gv[:, k, 0:D],
                        start=(n_done == 0),
                        stop=(n_done == n_slots - 1),
                    )
                    n_done += 1

            # rep = acc / len  (ScalarE: PSUM -> SBUF with per-partition scale)
            rep = spool.tile([P, D], F32, tag="rep")
            nc.scalar.mul(rep[:], acc[:], inv_t[:, :1])

            # repT chunks + first MLP layer: h = relu(rep @ W1 + b1), as [H, P]
            h_psum = psum_pool.tile([H, P], F32, tag="h")
            for j, (d0, dc) in enumerate(D_CHUNKS):
                tp = psum_pool.tile([P, P], F32, tag="tp")
                nc.tensor.transpose(tp[:dc, :], rep[:, d0:d0 + dc], ident[:])
                repT = spool.tile([P, P], F32, tag="repT")
                nc.vector.tensor_copy(repT[:dc, :], tp[:dc, :])
                nc.tensor.matmul(
                    out=h_psum[:],
                    lhsT=w1_sb[:dc, j * H:(j + 1) * H],
                    rhs=repT[:dc, :],
                    start=(j == 0),
                    stop=(j == len(D_CHUNKS) - 1),
                )
            h_sb = spool.tile([H, P], F32, tag="hsb")
            nc.scalar.activation(
                h_sb[:], h_psum[:], mybir.ActivationFunctionType.Relu,
                bias=b1_sb[:, :1], scale=1.0,
            )

            # logits = h @ W2 + b2, as [C, P]
            o_psum = psum_pool.tile([C, P], F32, tag="o")
            nc.tensor.matmul(out=o_psum[:], lhsT=w2_sb[:], rhs=h_sb[:],
                             start=True, stop=True)
            logits_sb = spool.tile([C, P], F32, tag="lg")
            nc.scalar.activation(
                logits_sb[:], o_psum[:], mybir.ActivationFunctionType.Identity,
                bias=b2_sb[:, :1], scale=1.0,
            )
            nc.sync.dma_start(out_dram[:, r0:r0 + P], logits_sb[:])

    nc.finalize()
    return nc


def _wrap_idx(block):
    """[P, C] token-slot indices -> [P, 8*C] wrapped+replicated int16 tile.

    dma_gather maps flat index j -> partition j%128, column-group j//128, and
    reads the flat list wrapped over 16 partitions (element j at partition
    j%16, column j//16), replicated across the eight 16-partition groups.
    """
    p, c = block.shape
    flat = block.T.reshape(-1)              # j = col*128 + p order
    w = flat.reshape(8 * c, 16).T           # [16, 8*c]: element j at (j%16, j//16)
    return np.tile(w, (8, 1))               # replicate to 128 partitions


def _prep_idx(x32):
    """Split tokens lo/hi per row, pad with fillers, build wrapped idx tiles.

    Returns (idx arrays per core [N_TILES, P, 8*(k_lo+k_hi)], k_lo, k_hi).
    """
    is_hi = x32 >= X_SPLIT
    n_lo = (~is_hi).sum(axis=1)                      # [B]
    k_lo = int(n_lo.max())
    k_hi = int((L - n_lo).max())
    order = np.argsort(is_hi, axis=1, kind="stable")  # lo positions first
    xo = np.take_along_axis(x32, order, axis=1)       # [B, L] lo tokens then hi

    cols = np.arange(L)[None, :]
    lo_vals = np.where(cols < n_lo[:, None], xo + 1, LO_FILL)
    # hi tokens start at column n_lo[p]
    hi_src = np.take_along_axis(
        xo, np.minimum(cols + n_lo[:, None], L - 1), axis=1)
    hi_vals = np.where(cols < (L - n_lo)[:, None], hi_src - 17233, HI_FILL)

    lo16 = lo_vals[:, :k_lo].astype(np.int16)
    hi16 = hi_vals[:, :k_hi].astype(np.int16)

    idx_per_core = []
    for c in range(N_CORES):
        tiles = []
        for t in range(N_TILES):
            r0 = c * BS + t * P
            blocks = []
            for c0 in range(0, k_lo, CHUNK):
                blocks.append(_wrap_idx(lo16[r0:r0 + P, c0:min(c0 + CHUNK, k_lo)]))
            for c0 in range(0, k_hi, CHUNK):
                blocks.append(_wrap_idx(hi16[r0:r0 + P, c0:min(c0 + CHUNK, k_hi)]))
            tiles.append(np.concatenate(blocks, axis=1))
        idx_per_core.append(np.ascontiguousarray(np.stack(tiles)))
    return idx_per_core, k_lo, k_hi


def _prep_inputs(x, lengths, emb_table, W1, b1, W2, b2):
    x32 = np.asarray(x).astype(np.int32)
    idx_per_core, k_lo, k_hi = _prep_idx(x32)

    lens = np.ascontiguousarray(
        np.asarray(lengths).astype(np.float32).reshape(B, 1))
    emb_p = np.zeros((V + 2, E), dtype=np.float32)
    emb_p[1:V + 1, :D] = np.asarray(emb_table, dtype=np.float32)
    w1 = np.ascontiguousarray(np.asarray(W1, dtype=np.float32))
    b1c = np.ascontiguousarray(np.asarray(b1, dtype=np.float32).reshape(H, 1))
    w2 = np.ascontiguousarray(np.asarray(W2, dtype=np.float32))
    b2c = np.ascontiguousarray(np.asarray(b2, dtype=np.float32).reshape(C, 1))
    in_maps = [
        {
            "idx": idx_per_core[c],
            "lens": lens[c * BS:(c + 1) * BS],
            "emb": emb_p,
            "w1": w1,
            "b1": b1c,
            "w2": w2,
            "b2": b2c,
        }
        for c in range(N_CORES)
    ]
    return in_maps, k_lo, k_hi


def run_on_device(in_maps, k_lo, k_hi, **kwargs):
    key = (k_lo, k_hi)
    if _CACHE.get("key") != key:
        _CACHE["nc"] = _build_nc(k_lo, k_hi)
        _CACHE["key"] = key
    return run_bass_kernel_spmd(_CACHE["nc"], in_maps, list(range(N_CORES)),
                                **kwargs)


def kernel(x, lengths, emb_table, W1, b1, W2, b2):
    in_maps, k_lo, k_hi = _prep_inputs(x, lengths, emb_table, W1, b1, W2, b2)
    res = run_on_device(in_maps, k_lo, k_hi)
    out = np.concatenate([r["out"] for r in res.results], axis=1)  # [C, B]
    return np.ascontiguousarray(out.T).astype(np.float32)  # [B, C]


# revision 18
# speedup vs baseline: 195.8619x; 1.1300x over previous
"""Trainium2 Bass kernel for BaselineDNN: embedding gather + length-normalized
sum-pool over L tokens + 2-layer MLP.

  logits[b] = relu((sum_l emb[x[b,l]]) / len[b] @ W1 + b1) @ W2 + b2

Sharding: data-parallel over batch. Each of the 8 cores handles B/8 = 256
batch rows; the embedding table and tiny MLP weights are replicated.

Gather strategy: the HW dma_gather primitive takes int16 (signed) row
indices, so a 50000-row table is unreachable in one gather. The table is
padded/shifted on host into [50002, 320] f32 (row 0 = zeros, rows
1..50000 = emb, row 50001 = zeros) and every token is routed to one of two
base views:
  lo: rows [0, 32768)      idx = x + 1       (x <= 32766), filler idx 0
  hi: rows [17234, 50002)  idx = x - 17233   (x >= 32767), filler idx 32767
Per batch row the 200 tokens are partitioned (host-side, order-invariant
under the sum) into a lo-list and a hi-list, padded with zero-row fillers
to the global maxima K_LO / K_HI. Each 128-row batch tile then issues
ceil(K_LO/C) + ceil(K_HI/C) rectangular dma_gathers of up to C=50 tokens
per row (one 1280B row per index).

Pooling: per gathered token-slot, a TensorE matmul with a 128x128 identity
as the stationary operand accumulates the [128, 300] slot slice into PSUM
(acc += I.T @ g_slot). Then ScalarE scales by 1/len, TensorE transposes
rep and runs the two MLP layers, ScalarE applies bias/relu. Logits are
written transposed [3, 256] per core; the host reassembles [2048, 3].
"""

import numpy as np
from contextlib import ExitStack

import concourse.bass as bass
import concourse.bacc as bacc
import concourse.mybir as mybir
import concourse.tile as tile
from concourse.bass_utils import run_bass_kernel_spmd
from concourse.masks import make_identity

# Problem shapes (hardcoded per spec)
B, L, V, D, H, C = 2048, 200, 50000, 300, 32, 3
N_CORES = 8
BS = B // N_CORES   # 256 batch rows per core
P = 128             # partitions
N_TILES = BS // P   # batch tiles per core
E = 384             # padded fp16 embedding row (768B, multiple of 256B)
CHUNK = 25          # max tokens per dma_gather (128*25=3200 descriptors
                    # fits the per-engine SWDGE ring: ~256 descs x 16 engines)
D_CHUNKS = [(0, 128), (128, 128), (256, 44)]  # D=300 split for transposes

LO_BASE = 0         # lo view: table rows [0, 32768)
HI_BASE = 17234     # hi view: table rows [17234, 50002)
LO_FILL = 0         # zero row (table row 0)
HI_FILL = 32767     # zero row (table row 50001)
X_SPLIT = 32767     # x < split -> lo (idx x+1); x >= split -> hi (idx x-17233)

F32 = mybir.dt.float32
F16 = mybir.dt.float16
I16 = mybir.dt.int16

_CACHE = {}


def _chunks(total):
    out = []
    while total > 0:
        c = min(CHUNK, total)
        out.append(c)
        total -= c
    return out


def _build_nc(k_lo, k_hi, reps=1):
    lo_chunks = _chunks(k_lo)
    hi_chunks = _chunks(k_hi)
    n_slots = k_lo + k_hi
    idx_cols = 8 * n_slots  # wrapped idx layout: 16 rows per 128 indices

    # 4 SWDGE queues: a single queue serializes gathers on per-queue ring
    # bookkeeping (await prior DMA completion); round-robin over 4 queues
    # keeps descriptor-gen and transfers pipelined at HBM bandwidth.
    nc = bacc.Bacc("TRN2", debug=False, num_devices=N_CORES,
                   num_swdge_queues=4, dynamic_dma_scratch_size=32768)

    idx_in = nc.declare_dram_parameter("idx", [N_TILES, P, idx_cols], I16,
                                       isOutput=False)
    len_in = nc.declare_dram_parameter("lens", [BS, 1], F32, isOutput=False)
    emb_in = nc.declare_dram_parameter("emb", [V + 2, E], F16, isOutput=False)
    w1_in = nc.declare_dram_parameter("w1", [D, H], F32, isOutput=False)
    b1_in = nc.declare_dram_parameter("b1", [H, 1], F32, isOutput=False)
    w2_in = nc.declare_dram_parameter("w2", [H, C], F32, isOutput=False)
    b2_in = nc.declare_dram_parameter("b2", [C, 1], F32, isOutput=False)
    out_dram = nc.declare_dram_parameter("out", [C, BS], F32, isOutput=True)

    emb_lo = emb_in[LO_BASE:LO_BASE + 32768, :]
    emb_hi = emb_in[HI_BASE:HI_BASE + 32768, :]

    with tile.TileContext(nc) as tc, ExitStack() as ctx:
        const_pool = ctx.enter_context(tc.tile_pool(name="const", bufs=1))
        xpool = ctx.enter_context(tc.tile_pool(name="xp", bufs=2))
        gpool = ctx.enter_context(tc.tile_pool(name="gp", bufs=4))
        spool = ctx.enter_context(tc.tile_pool(name="sp", bufs=2))
        psum_pool = ctx.enter_context(tc.tile_pool(name="ps", bufs=2, space="PSUM"))
        psum_acc = ctx.enter_context(tc.tile_pool(name="psacc", bufs=2, space="PSUM"))

        ident = const_pool.tile([P, P], F32)
        make_identity(nc, ident[:])
        ident16 = const_pool.tile([P, P], F16)
        make_identity(nc, ident16[:])
        w1_sb = const_pool.tile([P, 3 * H], F32)  # chunk j at cols [j*H, (j+1)*H)
        for j, (d0, dc) in enumerate(D_CHUNKS):
            nc.sync.dma_start(w1_sb[:dc, j * H:(j + 1) * H], w1_in[d0:d0 + dc, :])
        b1_sb = const_pool.tile([H, 1], F32)
        nc.sync.dma_start(b1_sb[:], b1_in[:])
        w2_sb = const_pool.tile([H, C], F32)
        nc.sync.dma_start(w2_sb[:], w2_in[:])
        b2_sb = const_pool.tile([C, 1], F32)
        nc.sync.dma_start(b2_sb[:], b2_in[:])

        loop_ctx = tc.For_i(0, reps, 1) if reps > 1 else None
        if loop_ctx is not None:
            ctx.enter_context(loop_ctx)

        for t in range(N_TILES):
            r0 = t * P
            idx_t = xpool.tile([P, idx_cols], I16, tag="xt")
            nc.sync.dma_start(idx_t[:], idx_in[t, :, :])
            lens_t = xpool.tile([P, 1], F32, tag="lt")
            nc.sync.dma_start(lens_t[:], len_in[r0:r0 + P, :])
            inv_t = xpool.tile([P, 1], F32, tag="it")
            nc.vector.reciprocal(inv_t[:], lens_t[:])

            acc = psum_acc.tile([P, D], F32)
            n_done = 0  # token slots accumulated so far
            col0 = 0    # idx column offset
            plan = [(c, emb_lo) for c in lo_chunks] + [(c, emb_hi) for c in hi_chunks]
            for ci, (c, src) in enumerate(plan):
                g = gpool.tile([P, CHUNK * E], F16, tag="g")
                gv = g[:, :c * E].rearrange("p (c e) -> p c e", c=c, e=E)
                nc.gpsimd.dma_gather(
                    out_ap=gv,
                    in_ap=src,
                    idxs_ap=idx_t[:, col0:col0 + 8 * c],
                    num_idxs=P * c,
                    num_idxs_reg=P * c,
                    elem_size=E,
                    # >64 descriptors per engine: must not coalesce the whole
                    # stream into one SDMA packet (64-descriptor HW ceiling)
                    single_packet=False,
                    queue_num=(t * len(plan) + ci) % 4,
                )
                col0 += 8 * c
                for k in range(c):
                    nc.tensor.matmul(
                        out=acc[:],
                        lhsT=ident16[:],
                        rhs=gv[:, k, 0:D],
                        start=(n_done == 0),
                        stop=(n_done == n_slots - 1),
                    )
                    n_done += 1

            # rep = acc / len  (ScalarE: PSUM -> SBUF with per-partition scale)
            rep = spool.tile([P, D], F32, tag="rep")
            nc.scalar.mul(rep[:], acc[:], inv_t[:, :1])

            # repT chunks + first MLP layer: h = relu(rep @ W1 + b1), as [H, P]
            h_psum = psum_pool.tile([H, P], F32, tag="h")
            for j, (d0, dc) in enumerate(D_CHUNKS):
                tp = psum_pool.tile([P, P], F32, tag="tp")
                nc.tensor.transpose(tp[:dc, :], rep[:, d0:d0 + dc], ident[:])
                repT = spool.tile([P, P], F32, tag="repT")
                nc.vector.tensor_copy(repT[:dc, :], tp[:dc, :])
                nc.tensor.matmul(
                    out=h_psum[:],
                    lhsT=w1_sb[:dc, j * H:(j + 1) * H],
                    rhs=repT[:dc, :],
                    start=(j == 0),
                    stop=(j == len(D_CHUNKS) - 1),
                )
            h_sb = spool.tile([H, P], F32, tag="hsb")
            nc.scalar.activation(
                h_sb[:], h_psum[:], mybir.ActivationFunctionType.Relu,
                bias=b1_sb[:, :1], scale=1.0,
            )

            # logits = h @ W2 + b2, as [C, P]
            o_psum = psum_pool.tile([C, P], F32, tag="o")
            nc.tensor.matmul(out=o_psum[:], lhsT=w2_sb[:], rhs=h_sb[:],
                             start=True, stop=True)
            logits_sb = spool.tile([C, P], F32, tag="lg")
            nc.scalar.activation(
                logits_sb[:], o_psum[:], mybir.ActivationFunctionType.Identity,
                bias=b2_sb[:, :1], scale=1.0,
            )
            nc.sync.dma_start(out_dram[:, r0:r0 + P], logits_sb[:])

    nc.finalize()
    return nc


def _wrap_idx(block):
    """[P, C] token-slot indices -> [P, 8*C] wrapped+replicated int16 tile.

    dma_gather maps flat index j -> partition j%128, column-group j//128, and
    reads the flat list wrapped over 16 partitions (element j at partition
    j%16, column j//16), replicated across the eight 16-partition groups.
    """
    p, c = block.shape
    flat = block.T.reshape(-1)              # j = col*128 + p order
    w = flat.reshape(8 * c, 16).T           # [16, 8*c]: element j at (j%16, j//16)
    return np.tile(w, (8, 1))               # replicate to 128 partitions


def _prep_idx(x32):
    """Split tokens lo/hi per row, pad with fillers, build wrapped idx tiles.

    Returns (idx arrays per core [N_TILES, P, 8*(k_lo+k_hi)], k_lo, k_hi).
    """
    is_hi = x32 >= X_SPLIT
    n_lo = (~is_hi).sum(axis=1)                      # [B]
    k_lo = int(n_lo.max())
    k_hi = int((L - n_lo).max())
    order = np.argsort(is_hi, axis=1, kind="stable")  # lo positions first
    xo = np.take_along_axis(x32, order, axis=1)       # [B, L] lo tokens then hi

    cols = np.arange(L)[None, :]
    lo_vals = np.where(cols < n_lo[:, None], xo + 1, LO_FILL)
    # hi tokens start at column n_lo[p]
    hi_src = np.take_along_axis(
        xo, np.minimum(cols + n_lo[:, None], L - 1), axis=1)
    hi_vals = np.where(cols < (L - n_lo)[:, None], hi_src - 17233, HI_FILL)

    lo16 = lo_vals[:, :k_lo].astype(np.int16)
    hi16 = hi_vals[:, :k_hi].astype(np.int16)

    idx_per_core = []
    for c in range(N_CORES):
        tiles = []
        for t in range(N_TILES):
            r0 = c * BS + t * P
            blocks = []
            for c0 in range(0, k_lo, CHUNK):
                blocks.append(_wrap_idx(lo16[r0:r0 + P, c0:min(c0 + CHUNK, k_lo)]))
            for c0 in range(0, k_hi, CHUNK):
                blocks.append(_wrap_idx(hi16[r0:r0 + P, c0:min(c0 + CHUNK, k_hi)]))
            tiles.append(np.concatenate(blocks, axis=1))
        idx_per_core.append(np.ascontiguousarray(np.stack(tiles)))
    return idx_per_core, k_lo, k_hi


def _prep_inputs(x, lengths, emb_table, W1, b1, W2, b2):
    x32 = np.asarray(x).astype(np.int32)
    idx_per_core, k_lo, k_hi = _prep_idx(x32)

    lens = np.ascontiguousarray(
        np.asarray(lengths).astype(np.float32).reshape(B, 1))
    emb_p = np.zeros((V + 2, E), dtype=np.float16)
    emb_p[1:V + 1, :D] = np.asarray(emb_table, dtype=np.float32).astype(np.float16)
    w1 = np.ascontiguousarray(np.asarray(W1, dtype=np.float32))
    b1c = np.ascontiguousarray(np.asarray(b1, dtype=np.float32).reshape(H, 1))
    w2 = np.ascontiguousarray(np.asarray(W2, dtype=np.float32))
    b2c = np.ascontiguousarray(np.asarray(b2, dtype=np.float32).reshape(C, 1))
    in_maps = [
        {
            "idx": idx_per_core[c],
            "lens": lens[c * BS:(c + 1) * BS],
            "emb": emb_p,
            "w1": w1,
            "b1": b1c,
            "w2": w2,
            "b2": b2c,
        }
        for c in range(N_CORES)
    ]
    return in_maps, k_lo, k_hi


def run_on_device(in_maps, k_lo, k_hi, **kwargs):
    key = (k_lo, k_hi)
    if _CACHE.get("key") != key:
        _CACHE["nc"] = _build_nc(k_lo, k_hi)
        _CACHE["key"] = key
    return run_bass_kernel_spmd(_CACHE["nc"], in_maps, list(range(N_CORES)),
                                **kwargs)


def kernel(x, lengths, emb_table, W1, b1, W2, b2):
    in_maps, k_lo, k_hi = _prep_inputs(x, lengths, emb_table, W1, b1, W2, b2)
    res = run_on_device(in_maps, k_lo, k_hi)
    out = np.concatenate([r["out"] for r in res.results], axis=1)  # [C, B]
    return np.ascontiguousarray(out.T).astype(np.float32)  # [B, C]


# revision 19
# speedup vs baseline: 223.8890x; 1.1431x over previous
"""Trainium2 Bass kernel for BaselineDNN: embedding gather + length-normalized
sum-pool over L tokens + 2-layer MLP.

  logits[b] = relu((sum_l emb[x[b,l]]) / len[b] @ W1 + b1) @ W2 + b2

Sharding: data-parallel over batch. Each of the 8 cores handles B/8 = 256
batch rows; the embedding table and tiny MLP weights are replicated.

Gather strategy: the HW dma_gather primitive takes int16 (signed) row
indices, so a 50000-row table is unreachable in one gather. The table is
padded/shifted on host into [50002, 320] f32 (row 0 = zeros, rows
1..50000 = emb, row 50001 = zeros) and every token is routed to one of two
base views:
  lo: rows [0, 32768)      idx = x + 1       (x <= 32766), filler idx 0
  hi: rows [17234, 50002)  idx = x - 17233   (x >= 32767), filler idx 32767
Per batch row the 200 tokens are partitioned (host-side, order-invariant
under the sum) into a lo-list and a hi-list, padded with zero-row fillers
to the global maxima K_LO / K_HI. Each 128-row batch tile then issues
ceil(K_LO/C) + ceil(K_HI/C) rectangular dma_gathers of up to C=50 tokens
per row (one 1280B row per index).

Pooling: per gathered token-slot, a TensorE matmul with a 128x128 identity
as the stationary operand accumulates the [128, 300] slot slice into PSUM
(acc += I.T @ g_slot). Then ScalarE scales by 1/len, TensorE transposes
rep and runs the two MLP layers, ScalarE applies bias/relu. Logits are
written transposed [3, 256] per core; the host reassembles [2048, 3].
"""

import numpy as np
from contextlib import ExitStack

import concourse.bass as bass
import concourse.bacc as bacc
import concourse.mybir as mybir
import concourse.tile as tile
from concourse.bass_utils import run_bass_kernel_spmd
from concourse.masks import make_identity

# Problem shapes (hardcoded per spec)
B, L, V, D, H, C = 2048, 200, 50000, 300, 32, 3
N_CORES = 8
BS = B // N_CORES   # 256 batch rows per core
P = 128             # partitions
N_TILES = BS // P   # batch tiles per core
E = 384             # padded fp16 embedding row (768B, multiple of 256B)
CHUNK = 50          # max tokens per dma_gather: 128*50=6400 descriptors
                    # = 401/engine, fits the 512-desc/engine ring
                    # (dynamic_dma_scratch_size 32768 / 64B)
D_CHUNKS = [(0, 128), (128, 128), (256, 44)]  # D=300 split for transposes

LO_BASE = 0         # lo view: table rows [0, 32768)
HI_BASE = 17234     # hi view: table rows [17234, 50002)
LO_FILL = 0         # zero row (table row 0)
HI_FILL = 32767     # zero row (table row 50001)
X_SPLIT = 32767     # x < split -> lo (idx x+1); x >= split -> hi (idx x-17233)

F32 = mybir.dt.float32
F16 = mybir.dt.float16
I16 = mybir.dt.int16

_CACHE = {}


def _chunks(total):
    out = []
    while total > 0:
        c = min(CHUNK, total)
        out.append(c)
        total -= c
    return out


def _build_nc(k_lo, k_hi, reps=1):
    lo_chunks = _chunks(k_lo)
    hi_chunks = _chunks(k_hi)
    n_slots = k_lo + k_hi
    idx_cols = 8 * n_slots  # wrapped idx layout: 16 rows per 128 indices

    # 4 SWDGE queues: a single queue serializes gathers on per-queue ring
    # bookkeeping (await prior DMA completion); round-robin over 4 queues
    # keeps descriptor-gen and transfers pipelined at HBM bandwidth.
    nc = bacc.Bacc("TRN2", debug=False, num_devices=N_CORES,
                   num_swdge_queues=4, dynamic_dma_scratch_size=32768)

    idx_in = nc.declare_dram_parameter("idx", [N_TILES, P, idx_cols], I16,
                                       isOutput=False)
    len_in = nc.declare_dram_parameter("lens", [BS, 1], F32, isOutput=False)
    emb_in = nc.declare_dram_parameter("emb", [V + 2, E], F16, isOutput=False)
    w1_in = nc.declare_dram_parameter("w1", [D, H], F32, isOutput=False)
    b1_in = nc.declare_dram_parameter("b1", [H, 1], F32, isOutput=False)
    w2_in = nc.declare_dram_parameter("w2", [H, C], F32, isOutput=False)
    b2_in = nc.declare_dram_parameter("b2", [C, 1], F32, isOutput=False)
    out_dram = nc.declare_dram_parameter("out", [C, BS], F32, isOutput=True)

    emb_lo = emb_in[LO_BASE:LO_BASE + 32768, :]
    emb_hi = emb_in[HI_BASE:HI_BASE + 32768, :]

    with tile.TileContext(nc) as tc, ExitStack() as ctx:
        const_pool = ctx.enter_context(tc.tile_pool(name="const", bufs=1))
        xpool = ctx.enter_context(tc.tile_pool(name="xp", bufs=2))
        gpool = ctx.enter_context(tc.tile_pool(name="gp", bufs=4))
        spool = ctx.enter_context(tc.tile_pool(name="sp", bufs=2))
        psum_pool = ctx.enter_context(tc.tile_pool(name="ps", bufs=2, space="PSUM"))
        psum_acc = ctx.enter_context(tc.tile_pool(name="psacc", bufs=2, space="PSUM"))

        ident = const_pool.tile([P, P], F32)
        make_identity(nc, ident[:])
        ident16 = const_pool.tile([P, P], F16)
        make_identity(nc, ident16[:])
        w1_sb = const_pool.tile([P, 3 * H], F32)  # chunk j at cols [j*H, (j+1)*H)
        for j, (d0, dc) in enumerate(D_CHUNKS):
            nc.sync.dma_start(w1_sb[:dc, j * H:(j + 1) * H], w1_in[d0:d0 + dc, :])
        b1_sb = const_pool.tile([H, 1], F32)
        nc.sync.dma_start(b1_sb[:], b1_in[:])
        w2_sb = const_pool.tile([H, C], F32)
        nc.sync.dma_start(w2_sb[:], w2_in[:])
        b2_sb = const_pool.tile([C, 1], F32)
        nc.sync.dma_start(b2_sb[:], b2_in[:])

        loop_ctx = tc.For_i(0, reps, 1) if reps > 1 else None
        if loop_ctx is not None:
            ctx.enter_context(loop_ctx)

        for t in range(N_TILES):
            r0 = t * P
            idx_t = xpool.tile([P, idx_cols], I16, tag="xt")
            nc.sync.dma_start(idx_t[:], idx_in[t, :, :])
            lens_t = xpool.tile([P, 1], F32, tag="lt")
            nc.sync.dma_start(lens_t[:], len_in[r0:r0 + P, :])
            inv_t = xpool.tile([P, 1], F32, tag="it")
            nc.vector.reciprocal(inv_t[:], lens_t[:])

            acc = psum_acc.tile([P, D], F32)
            n_done = 0  # token slots accumulated so far
            col0 = 0    # idx column offset
            plan = [(c, emb_lo) for c in lo_chunks] + [(c, emb_hi) for c in hi_chunks]
            for ci, (c, src) in enumerate(plan):
                g = gpool.tile([P, CHUNK * E], F16, tag="g")
                gv = g[:, :c * E].rearrange("p (c e) -> p c e", c=c, e=E)
                nc.gpsimd.dma_gather(
                    out_ap=gv,
                    in_ap=src,
                    idxs_ap=idx_t[:, col0:col0 + 8 * c],
                    num_idxs=P * c,
                    num_idxs_reg=P * c,
                    elem_size=E,
                    # >64 descriptors per engine: must not coalesce the whole
                    # stream into one SDMA packet (64-descriptor HW ceiling)
                    single_packet=False,
                    queue_num=(t * len(plan) + ci) % 4,
                )
                col0 += 8 * c
                for k in range(c):
                    nc.tensor.matmul(
                        out=acc[:],
                        lhsT=ident16[:],
                        rhs=gv[:, k, 0:D],
                        start=(n_done == 0),
                        stop=(n_done == n_slots - 1),
                    )
                    n_done += 1

            # rep = acc / len  (ScalarE: PSUM -> SBUF with per-partition scale)
            rep = spool.tile([P, D], F32, tag="rep")
            nc.scalar.mul(rep[:], acc[:], inv_t[:, :1])

            # repT chunks + first MLP layer: h = relu(rep @ W1 + b1), as [H, P]
            h_psum = psum_pool.tile([H, P], F32, tag="h")
            for j, (d0, dc) in enumerate(D_CHUNKS):
                tp = psum_pool.tile([P, P], F32, tag="tp")
                nc.tensor.transpose(tp[:dc, :], rep[:, d0:d0 + dc], ident[:])
                repT = spool.tile([P, P], F32, tag="repT")
                nc.vector.tensor_copy(repT[:dc, :], tp[:dc, :])
                nc.tensor.matmul(
                    out=h_psum[:],
                    lhsT=w1_sb[:dc, j * H:(j + 1) * H],
                    rhs=repT[:dc, :],
                    start=(j == 0),
                    stop=(j == len(D_CHUNKS) - 1),
                )
            h_sb = spool.tile([H, P], F32, tag="hsb")
            nc.scalar.activation(
                h_sb[:], h_psum[:], mybir.ActivationFunctionType.Relu,
                bias=b1_sb[:, :1], scale=1.0,
            )

            # logits = h @ W2 + b2, as [C, P]
            o_psum = psum_pool.tile([C, P], F32, tag="o")
            nc.tensor.matmul(out=o_psum[:], lhsT=w2_sb[:], rhs=h_sb[:],
                             start=True, stop=True)
            logits_sb = spool.tile([C, P], F32, tag="lg")
            nc.scalar.activation(
                logits_sb[:], o_psum[:], mybir.ActivationFunctionType.Identity,
                bias=b2_sb[:, :1], scale=1.0,
            )
            nc.sync.dma_start(out_dram[:, r0:r0 + P], logits_sb[:])

    nc.finalize()
    return nc


def _wrap_idx(block):
    """[P, C] token-slot indices -> [P, 8*C] wrapped+replicated int16 tile.

    dma_gather maps flat index j -> partition j%128, column-group j//128, and
    reads the flat list wrapped over 16 partitions (element j at partition
    j%16, column j//16), replicated across the eight 16-partition groups.
    """
    p, c = block.shape
    flat = block.T.reshape(-1)              # j = col*128 + p order
    w = flat.reshape(8 * c, 16).T           # [16, 8*c]: element j at (j%16, j//16)
    return np.tile(w, (8, 1))               # replicate to 128 partitions


def _prep_idx(x32):
    """Split tokens lo/hi per row, pad with fillers, build wrapped idx tiles.

    Returns (idx arrays per core [N_TILES, P, 8*(k_lo+k_hi)], k_lo, k_hi).
    """
    is_hi = x32 >= X_SPLIT
    n_lo = (~is_hi).sum(axis=1)                      # [B]
    k_lo = int(n_lo.max())
    k_hi = int((L - n_lo).max())
    order = np.argsort(is_hi, axis=1, kind="stable")  # lo positions first
    xo = np.take_along_axis(x32, order, axis=1)       # [B, L] lo tokens then hi

    cols = np.arange(L)[None, :]
    lo_vals = np.where(cols < n_lo[:, None], xo + 1, LO_FILL)
    # hi tokens start at column n_lo[p]
    hi_src = np.take_along_axis(
        xo, np.minimum(cols + n_lo[:, None], L - 1), axis=1)
    hi_vals = np.where(cols < (L - n_lo)[:, None], hi_src - 17233, HI_FILL)

    lo16 = lo_vals[:, :k_lo].astype(np.int16)
    hi16 = hi_vals[:, :k_hi].astype(np.int16)

    idx_per_core = []
    for c in range(N_CORES):
        tiles = []
        for t in range(N_TILES):
            r0 = c * BS + t * P
            blocks = []
            for c0 in range(0, k_lo, CHUNK):
                blocks.append(_wrap_idx(lo16[r0:r0 + P, c0:min(c0 + CHUNK, k_lo)]))
            for c0 in range(0, k_hi, CHUNK):
                blocks.append(_wrap_idx(hi16[r0:r0 + P, c0:min(c0 + CHUNK, k_hi)]))
            tiles.append(np.concatenate(blocks, axis=1))
        idx_per_core.append(np.ascontiguousarray(np.stack(tiles)))
    return idx_per_core, k_lo, k_hi


def _prep_inputs(x, lengths, emb_table, W1, b1, W2, b2):
    x32 = np.asarray(x).astype(np.int32)
    idx_per_core, k_lo, k_hi = _prep_idx(x32)

    lens = np.ascontiguousarray(
        np.asarray(lengths).astype(np.float32).reshape(B, 1))
    emb_p = np.zeros((V + 2, E), dtype=np.float16)
    emb_p[1:V + 1, :D] = np.asarray(emb_table, dtype=np.float32).astype(np.float16)
    w1 = np.ascontiguousarray(np.asarray(W1, dtype=np.float32))
    b1c = np.ascontiguousarray(np.asarray(b1, dtype=np.float32).reshape(H, 1))
    w2 = np.ascontiguousarray(np.asarray(W2, dtype=np.float32))
    b2c = np.ascontiguousarray(np.asarray(b2, dtype=np.float32).reshape(C, 1))
    in_maps = [
        {
            "idx": idx_per_core[c],
            "lens": lens[c * BS:(c + 1) * BS],
            "emb": emb_p,
            "w1": w1,
            "b1": b1c,
            "w2": w2,
            "b2": b2c,
        }
        for c in range(N_CORES)
    ]
    return in_maps, k_lo, k_hi


def run_on_device(in_maps, k_lo, k_hi, **kwargs):
    key = (k_lo, k_hi)
    if _CACHE.get("key") != key:
        _CACHE["nc"] = _build_nc(k_lo, k_hi)
        _CACHE["key"] = key
    return run_bass_kernel_spmd(_CACHE["nc"], in_maps, list(range(N_CORES)),
                                **kwargs)


def kernel(x, lengths, emb_table, W1, b1, W2, b2):
    in_maps, k_lo, k_hi = _prep_inputs(x, lengths, emb_table, W1, b1, W2, b2)
    res = run_on_device(in_maps, k_lo, k_hi)
    out = np.concatenate([r["out"] for r in res.results], axis=1)  # [C, B]
    return np.ascontiguousarray(out.T).astype(np.float32)  # [B, C]


# revision 26
# speedup vs baseline: 241.4439x; 1.0784x over previous
"""Trainium2 Bass kernel for BaselineDNN: embedding gather + length-normalized
sum-pool over L tokens + 2-layer MLP.

  logits[b] = relu((sum_l emb[x[b,l]]) / len[b] @ W1 + b1) @ W2 + b2

Sharding: data-parallel over batch. Each of the 8 cores handles B/8 = 256
batch rows; the embedding table (fp16, padded) and the tiny MLP weights are
replicated. One SPMD program runs on all 8 cores.

Gather: the dma_gather primitive takes int16 (signed) row indices, so the
50000-row table is unreachable in one address window. The table is
padded/shifted on host into [50002, 384] fp16 (row 0 = zeros, rows
1..50000 = emb, row 50001 = zeros; 384 fp16 = 768B, a multiple of the
required 256B) and every token is routed to one of two base views:
  lo: rows [0, 32768)      idx = x + 1       (x <= 32766), filler idx 0
  hi: rows [17234, 50002)  idx = x - 17233   (x >= 32767), filler idx 32767
Per batch row the 200 tokens are partitioned host-side (order-invariant
under the sum) into a lo-list and a hi-list. Rows are globally sorted by
lo-count into 16 narrow-spread tiles of 128 (tile g pairs with tile 15-g on
a core to balance work); the host inverse-permutes the output.

Each tile issues rectangular dma_gathers of up to CHUNK tokens/row (one
768B row per index, descriptors generated by the Q7 SWDGE). Index slots
past each (tile-slot, chunk)'s valid column count are -1: the Q7 trims
trailing negatives, so they cost no descriptors and no HBM traffic. The
valid count is equalized across the 8 cores (max, rounded up to a
128-index column) so it is a compile-time immediate and the pooling
matmul structure matches exactly; slots between a core's own tokens and
the equalized count point at a zero row.

Pooling: per valid token-slot, a TensorE matmul with a 128x128 fp16
identity as the stationary operand accumulates the [128, 300] slot slice
into fp32 PSUM (acc += I.T @ g_slot). ScalarE scales by 1/len (PSUM ->
SBUF), TensorE transposes rep and runs both MLP layers, ScalarE applies
bias/relu. Logits are written transposed [3, 256] per core; the host
reassembles and un-permutes [2048, 3].

Perf notes (measured on TRN2):
  - 4 SWDGE queues round-robin: a single queue serializes gathers on
    per-queue ring bookkeeping (~30us/gather); 4 queues pipeline
    descriptor-gen against transfers.
  - single_packet=False is required: >64 descriptors per engine must not
    be coalesced into one SDMA packet.
  - fp16 data path halves HBM traffic and enables fast PE weight loads.
"""

import numpy as np
from contextlib import ExitStack

import concourse.bass as bass
import concourse.bacc as bacc
import concourse.mybir as mybir
import concourse.tile as tile
from concourse.bass_utils import run_bass_kernel_spmd
from concourse.masks import make_identity

# Problem shapes (hardcoded per spec)
B, L, V, D, H, C = 2048, 200, 50000, 300, 32, 3
N_CORES = 8
BS = B // N_CORES   # 256 batch rows per core
P = 128             # partitions
N_TILES = BS // P   # batch tiles per core
E = 384             # padded fp16 embedding row (768B, multiple of 256B)
CHUNK = 50          # max tokens per dma_gather: 128*50=6400 descriptors
                    # = 401/engine, fits the 512-desc/engine SWDGE ring
                    # (dynamic_dma_scratch_size 32768 / 64B)
D_CHUNKS = [(0, 128), (128, 128), (256, 44)]  # D=300 split for transposes

LO_BASE = 0         # lo view: table rows [0, 32768)
HI_BASE = 17234     # hi view: table rows [17234, 50002)
LO_FILL = 0         # zero row (table row 0)
HI_FILL = 32767     # zero row (table row 50001)
X_SPLIT = 32767     # x < split -> lo (idx x+1); x >= split -> hi (idx x-17233)
FILL_SENTINEL = -9999

F32 = mybir.dt.float32
F16 = mybir.dt.float16
I16 = mybir.dt.int16
I32 = mybir.dt.int32

_CACHE = {}


def _chunks(total):
    out = []
    while total > 0:
        out.append(min(CHUNK, total))
        total -= CHUNK
    return out


def _build_nc(k_lo, k_hi, cols, reps=1):
    """cols[t][ci] = valid 128-index columns for tile-slot t, chunk ci
    (0 = chunk fully skipped). Identical across cores by construction."""
    lo_chunks = _chunks(k_lo)
    hi_chunks = _chunks(k_hi)
    chunk_sizes = lo_chunks + hi_chunks
    idx_cols = 8 * (k_lo + k_hi)

    # 4 SWDGE queues: a single queue serializes gathers on per-queue ring
    # bookkeeping; round-robin over 4 queues keeps descriptor-gen and
    # transfers pipelined.
    nc = bacc.Bacc("TRN2", debug=False, num_devices=N_CORES,
                   num_swdge_queues=4, dynamic_dma_scratch_size=32768)

    idx_in = nc.declare_dram_parameter("idx", [N_TILES, P, idx_cols], I16,
                                       isOutput=False)
    len_in = nc.declare_dram_parameter("lens", [BS, 1], F32, isOutput=False)
    emb_in = nc.declare_dram_parameter("emb", [V + 2, E], F16, isOutput=False)
    w1_in = nc.declare_dram_parameter("w1", [D, H], F32, isOutput=False)
    b1_in = nc.declare_dram_parameter("b1", [H, 1], F32, isOutput=False)
    w2_in = nc.declare_dram_parameter("w2", [H, C], F32, isOutput=False)
    b2_in = nc.declare_dram_parameter("b2", [C, 1], F32, isOutput=False)
    out_dram = nc.declare_dram_parameter("out", [C, BS], F32, isOutput=True)

    emb_lo = emb_in[LO_BASE:LO_BASE + 32768, :]
    emb_hi = emb_in[HI_BASE:HI_BASE + 32768, :]

    with tile.TileContext(nc) as tc, ExitStack() as ctx:
        const_pool = ctx.enter_context(tc.tile_pool(name="const", bufs=1))
        xpool = ctx.enter_context(tc.tile_pool(name="xp", bufs=2))
        gpool = ctx.enter_context(tc.tile_pool(name="gp", bufs=4))
        spool = ctx.enter_context(tc.tile_pool(name="sp", bufs=2))
        psum_pool = ctx.enter_context(tc.tile_pool(name="ps", bufs=2, space="PSUM"))
        psum_acc = ctx.enter_context(tc.tile_pool(name="psacc", bufs=2, space="PSUM"))

        ident = const_pool.tile([P, P], F32)
        make_identity(nc, ident[:])
        ident16 = const_pool.tile([P, P], F16)
        make_identity(nc, ident16[:])
        w1_sb = const_pool.tile([P, 3 * H], F32)  # chunk j at cols [j*H, (j+1)*H)
        for j, (d0, dc) in enumerate(D_CHUNKS):
            nc.sync.dma_start(w1_sb[:dc, j * H:(j + 1) * H], w1_in[d0:d0 + dc, :])
        b1_sb = const_pool.tile([H, 1], F32)
        nc.sync.dma_start(b1_sb[:], b1_in[:])
        w2_sb = const_pool.tile([H, C], F32)
        nc.sync.dma_start(w2_sb[:], w2_in[:])
        b2_sb = const_pool.tile([C, 1], F32)
        nc.sync.dma_start(b2_sb[:], b2_in[:])

        loop_ctx = tc.For_i(0, reps, 1) if reps > 1 else None
        if loop_ctx is not None:
            ctx.enter_context(loop_ctx)

        qn = 0  # gather queue round-robin
        for t in range(N_TILES):
            r0 = t * P
            idx_t = xpool.tile([P, idx_cols], I16, tag="xt")
            nc.sync.dma_start(idx_t[:], idx_in[t, :, :])
            lens_t = xpool.tile([P, 1], F32, tag="lt")
            nc.sync.dma_start(lens_t[:], len_in[r0:r0 + P, :])
            inv_t = xpool.tile([P, 1], F32, tag="it")
            nc.vector.reciprocal(inv_t[:], lens_t[:])

            n_valid_tot = sum(cols[t][ci] for ci in range(len(chunk_sizes)))
            acc = psum_acc.tile([P, D], F32)
            n_done = 0  # valid token-slot columns accumulated so far
            col0 = 0    # idx column offset
            srcs = [emb_lo] * len(lo_chunks) + [emb_hi] * len(hi_chunks)
            for ci, (c, src) in enumerate(zip(chunk_sizes, srcs)):
                ncols = cols[t][ci]
                if ncols > 0:
                    g = gpool.tile([P, CHUNK * E], F16, tag="g")
                    gv = g[:, :c * E].rearrange("p (c e) -> p c e", c=c, e=E)
                    nc.gpsimd.dma_gather(
                        out_ap=gv,
                        in_ap=src,
                        idxs_ap=idx_t[:, col0:col0 + 8 * c],
                        num_idxs=P * c,
                        # equalized post-trim count: the decode-side ring
                        # bookkeeping must match what the Q7 emits after
                        # trailing--1 trimming
                        num_idxs_reg=P * ncols,
                        elem_size=E,
                        # >64 descriptors/engine: must not coalesce the whole
                        # stream into one SDMA packet (64-descriptor ceiling)
                        single_packet=False,
                        queue_num=qn % 4,
                    )
                    qn += 1
                    for k in range(ncols):
                        nc.tensor.matmul(
                            out=acc[:],
                            lhsT=ident16[:],
                            rhs=gv[:, k, 0:D],
                            start=(n_done == 0),
                            stop=(n_done == n_valid_tot - 1),
                        )
                        n_done += 1
                col0 += 8 * c

            # rep = acc / len  (ScalarE: PSUM -> SBUF with per-partition scale)
            rep = spool.tile([P, D], F32, tag="rep")
            nc.scalar.mul(rep[:], acc[:], inv_t[:, :1])

            # repT chunks + first MLP layer: h = relu(rep @ W1 + b1), as [H, P]
            h_psum = psum_pool.tile([H, P], F32, tag="h")
            for j, (d0, dc) in enumerate(D_CHUNKS):
                tp = psum_pool.tile([P, P], F32, tag="tp")
                nc.tensor.transpose(tp[:dc, :], rep[:, d0:d0 + dc], ident[:])
                repT = spool.tile([P, P], F32, tag="repT")
                nc.vector.tensor_copy(repT[:dc, :], tp[:dc, :])
                nc.tensor.matmul(
                    out=h_psum[:],
                    lhsT=w1_sb[:dc, j * H:(j + 1) * H],
                    rhs=repT[:dc, :],
                    start=(j == 0),
                    stop=(j == len(D_CHUNKS) - 1),
                )
            h_sb = spool.tile([H, P], F32, tag="hsb")
            nc.scalar.activation(
                h_sb[:], h_psum[:], mybir.ActivationFunctionType.Relu,
                bias=b1_sb[:, :1], scale=1.0,
            )

            # logits = h @ W2 + b2, as [C, P]
            o_psum = psum_pool.tile([C, P], F32, tag="o")
            nc.tensor.matmul(out=o_psum[:], lhsT=w2_sb[:], rhs=h_sb[:],
                             start=True, stop=True)
            logits_sb = spool.tile([C, P], F32, tag="lg")
            nc.scalar.activation(
                logits_sb[:], o_psum[:], mybir.ActivationFunctionType.Identity,
                bias=b2_sb[:, :1], scale=1.0,
            )
            nc.sync.dma_start(out_dram[:, r0:r0 + P], logits_sb[:])

    nc.finalize()
    return nc


def _block_counts(vals, c0, c1):
    """Last real flat position + 1 for block columns [c0, c1) of `vals`
    ([P, K] with FILL_SENTINEL marking fillers), in j = col*128 + p order."""
    blk = vals[:, c0:c1]
    real = blk != FILL_SENTINEL
    if not real.any():
        return 0
    cc, pp = np.nonzero(real.T)
    return int((cc * P + pp).max() + 1)


def _wrap_block(blk, lead_fill, n_valid):
    """[P, C] block -> [P, 8*C] wrapped+replicated int16 idx tile.

    dma_gather maps flat index j -> partition j%128, column-group j//128,
    reading the flat list wrapped over 16 partitions (element j at partition
    j%16, column j//16), replicated across the eight 16-partition groups
    (each SWDGE queue's Q7 pair reads its own group).

    Positions < n_valid that are fillers point at a zero row; positions
    >= n_valid are -1 (trimmed by the Q7: no descriptors, no traffic).
    """
    p, c = blk.shape
    flat = blk.T.reshape(-1).astype(np.int32).copy()
    flat[flat == FILL_SENTINEL] = lead_fill
    flat[n_valid:] = -1
    flat = flat.astype(np.int16)
    w = flat.reshape(8 * c, 16).T           # [16, 8*c]: element j at (j%16, j//16)
    return np.tile(w, (8, 1))               # replicate to 128 partitions


def _prep_idx(x32):
    """Split tokens lo/hi per row, globally sort rows by lo-count into
    narrow-spread tiles, equalize per-(tile-slot, chunk) valid columns
    across cores, and build wrapped idx tiles.

    Returns (idx arrays per core [N_TILES, P, 8*(k_lo+k_hi)], k_lo, k_hi,
    cols, row_order) where row_order[c*BS + i] is the original batch row
    handled by core c, slot i.
    """
    is_hi = x32 >= X_SPLIT
    n_lo = (~is_hi).sum(axis=1)                       # [B]
    k_lo = int(n_lo.max())
    k_hi = int((L - n_lo).max())
    order = np.argsort(is_hi, axis=1, kind="stable")  # lo positions first
    xo = np.take_along_axis(x32, order, axis=1)       # [B, L] lo tokens then hi

    colsr = np.arange(L)[None, :]
    lo_vals = np.where(colsr < n_lo[:, None], xo + 1, FILL_SENTINEL)
    hi_src = np.take_along_axis(
        xo, np.minimum(colsr + n_lo[:, None], L - 1), axis=1)
    hi_vals = np.where(colsr < (L - n_lo)[:, None], hi_src - 17233, FILL_SENTINEL)
    lo_all = lo_vals[:, :k_lo]
    hi_all = hi_vals[:, :k_hi]

    # Global sort by n_lo -> 16 tiles of 128 rows with narrow n_lo spread;
    # within a tile sort descending so chunk tails are maximally trimmable.
    # Pair tile g with tile 15-g on one core to balance per-core work.
    gorder = np.argsort(n_lo, kind="stable")
    n_gtiles = B // P
    gtiles = [gorder[i * P:(i + 1) * P] for i in range(n_gtiles)]
    gtiles = [t[np.argsort(-n_lo[t], kind="stable")] for t in gtiles]

    lo_chunks = _chunks(k_lo)
    hi_chunks = _chunks(k_hi)
    bounds = []
    off = 0
    for c in lo_chunks:
        bounds.append(("lo", off, off + c))
        off += c
    off = 0
    for c in hi_chunks:
        bounds.append(("hi", off, off + c))
        off += c

    tile_rows = {}
    for c in range(N_CORES):
        for t, g in enumerate([c, n_gtiles - 1 - c]):
            tile_rows[(c, t)] = gtiles[g]

    # Equalized valid column counts per (tile-slot, chunk): max over cores,
    # rounded up to a full 128-index column so every gathered column is
    # completely written (pooling matmuls touch only valid columns).
    cols = []
    for t in range(N_TILES):
        tcols = []
        for (kind, c0, c1) in bounds:
            m = 0
            for c in range(N_CORES):
                rows = tile_rows[(c, t)]
                vals = lo_all[rows] if kind == "lo" else hi_all[rows]
                m = max(m, _block_counts(vals, c0, c1))
            tcols.append((m + P - 1) // P)
        cols.append(tuple(tcols))
    cols = tuple(cols)

    idx_per_core = []
    row_order = np.empty(B, dtype=np.int64)
    for c in range(N_CORES):
        tiles = []
        for t in range(N_TILES):
            rows = tile_rows[(c, t)]
            row_order[c * BS + t * P:c * BS + (t + 1) * P] = rows
            blocks = []
            for ci, (kind, c0, c1) in enumerate(bounds):
                vals = lo_all[rows] if kind == "lo" else hi_all[rows]
                fill = LO_FILL if kind == "lo" else HI_FILL
                blocks.append(_wrap_block(vals[:, c0:c1], fill,
                                          cols[t][ci] * P))
            tiles.append(np.concatenate(blocks, axis=1))
        idx_per_core.append(np.ascontiguousarray(np.stack(tiles)))
    return idx_per_core, k_lo, k_hi, cols, row_order


def _prep_inputs(x, lengths, emb_table, W1, b1, W2, b2):
    x32 = np.asarray(x).astype(np.int32)
    idx_per_core, k_lo, k_hi, cols, row_order = _prep_idx(x32)

    lens = np.ascontiguousarray(
        np.asarray(lengths).astype(np.float32).reshape(B, 1)[row_order])
    emb_p = np.zeros((V + 2, E), dtype=np.float16)
    emb_p[1:V + 1, :D] = np.asarray(emb_table, dtype=np.float32).astype(np.float16)
    w1 = np.ascontiguousarray(np.asarray(W1, dtype=np.float32))
    b1c = np.ascontiguousarray(np.asarray(b1, dtype=np.float32).reshape(H, 1))
    w2 = np.ascontiguousarray(np.asarray(W2, dtype=np.float32))
    b2c = np.ascontiguousarray(np.asarray(b2, dtype=np.float32).reshape(C, 1))
    in_maps = [
        {
            "idx": idx_per_core[c],
            "lens": lens[c * BS:(c + 1) * BS],
            "emb": emb_p,
            "w1": w1,
            "b1": b1c,
            "w2": w2,
            "b2": b2c,
        }
        for c in range(N_CORES)
    ]
    return in_maps, k_lo, k_hi, cols, row_order


def run_on_device(in_maps, k_lo, k_hi, cols, **kwargs):
    key = (k_lo, k_hi, cols)
    if _CACHE.get("key") != key:
        _CACHE["nc"] = _build_nc(k_lo, k_hi, cols)
        _CACHE["key"] = key
    return run_bass_kernel_spmd(_CACHE["nc"], in_maps, list(range(N_CORES)),
                                **kwargs)


def kernel(x, lengths, emb_table, W1, b1, W2, b2):
    in_maps, k_lo, k_hi, cols, row_order = _prep_inputs(
        x, lengths, emb_table, W1, b1, W2, b2)
    res = run_on_device(in_maps, k_lo, k_hi, cols)
    out = np.concatenate([r["out"] for r in res.results], axis=1)  # [C, B]
    full = np.empty((B, C), dtype=np.float32)
    full[row_order] = out.T  # undo the global row sort
    return full
